# revision 2
# baseline (speedup 1.0000x reference)
"""Trainium2 Bass kernel for nn_DetModel (conv heads + per-sample NMS).

Data-parallel over batch: 8 samples -> 8 NeuronCores. Each core runs
two conv3x3+conv3x3+1x1 branches (f32r matmuls), the small cls branch,
and an exact greedy NMS (rank -> scatter-sort -> chunked IoU ->
fixed-point suppression).
"""
import os
import numpy as np

P = 128
H = W = 256
R = 8
NBLK = H // R
Wp = W + 2
NBOX = 2048
NCH = NBOX // P
NITER = 6
SCORE_THR = 0.05

_BUILT = None


def _install_prof_shim():
    import sys
    import types

    if "antenv.axon_hooks" not in sys.modules:
        from trn_agent_boot.trn_boot import _ntff_profile_via_ctypes

        hook = _ntff_profile_via_ctypes("/opt/axon/libaxon_pjrt.so")
        mod = types.ModuleType("antenv.axon_hooks")
        mod._hook = hook
        mod.get_axon_ntff_profile_hook = lambda: mod._hook
        mod.set_axon_ntff_profile_hook = lambda h: setattr(mod, "_hook", h)
        sys.modules["antenv.axon_hooks"] = mod
    import concourse.bass_utils as bu

    bu.upload_artifacts = lambda tmpdir: f"local:{tmpdir}"


def _build():
    import concourse.bass as bass
    import concourse.bacc as bacc
    import concourse.mybir as mybir
    import concourse.tile as tile

    f32 = mybir.dt.float32
    f32r = mybir.dt.float32r
    bf16 = mybir.dt.bfloat16
    i32 = mybir.dt.int32
    Alu = mybir.AluOpType
    AX = mybir.AxisListType
    Act = mybir.ActivationFunctionType
    M01DT = mybir.dt.float8e4

    nc = bacc.Bacc("TRN2", target_bir_lowering=False, debug=False, num_devices=8)

    featT_d = nc.dram_tensor("featT", [P, H, W], f32r, kind="ExternalInput")
    fclsT_d = nc.dram_tensor("fclsT", [256, 16, 16], f32, kind="ExternalInput")
    wconv_d = nc.dram_tensor("wconv", [4, 9, P, P], f32r, kind="ExternalInput")
    bconv_d = nc.dram_tensor("bconv", [4, P], f32, kind="ExternalInput")
    w1x1_d = nc.dram_tensor("w1x1", [P, 5], f32r, kind="ExternalInput")
    wcls_d = nc.dram_tensor("wcls", [9, 2, P, P], f32, kind="ExternalInput")
    bcls_d = nc.dram_tensor("bcls", [P, 1], f32, kind="ExternalInput")
    fw1_d = nc.dram_tensor("fw1", [P, P], f32, kind="ExternalInput")
    fb1_d = nc.dram_tensor("fb1", [P, 1], f32, kind="ExternalInput")
    fw2_d = nc.dram_tensor("fw2", [P, P], f32, kind="ExternalInput")
    fb2_d = nc.dram_tensor("fb2", [P, 1], f32, kind="ExternalInput")
    ow_d = nc.dram_tensor("ow", [P, 3], f32, kind="ExternalInput")
    ob_d = nc.dram_tensor("ob", [3, 1], f32, kind="ExternalInput")
    i3_d = nc.dram_tensor("i3", [3, 3], f32, kind="ExternalInput")
    boxes_d = nc.dram_tensor("boxes", [NBOX, 4], f32, kind="ExternalInput")
    scores_d = nc.dram_tensor("scores", [1, NBOX], f32, kind="ExternalInput")
    tri_d = nc.dram_tensor("tri", [P, P], f32, kind="ExternalInput")
    trilo_d = nc.dram_tensor("trilo", [P, P], f32, kind="ExternalInput")

    scratch_d = nc.dram_tensor("scratch", [NBOX, 6], f32)

    oofs_d = nc.dram_tensor("oofs", [2, H * W], f32, kind="ExternalOutput")
    osz_d = nc.dram_tensor("osz", [2, H * W], f32, kind="ExternalOutput")
    owt_d = nc.dram_tensor("owt", [1, H * W], f32, kind="ExternalOutput")
    opred_d = nc.dram_tensor("opred", [1, 3], f32, kind="ExternalOutput")
    onmsb_d = nc.dram_tensor("onmsb", [NBOX, 4], f32, kind="ExternalOutput")
    onmss_d = nc.dram_tensor("onmss", [1, NBOX], f32, kind="ExternalOutput")

    NCHK = R * W // 512

    with tile.TileContext(nc) as tc:
        with (
            tc.tile_pool(name="wpool", bufs=1) as wpool,
            tc.tile_pool(name="conv", bufs=1) as conv,
            tc.tile_pool(name="cls", bufs=1) as clsp,
            tc.tile_pool(name="nbig", bufs=1) as nbig,
            tc.tile_pool(name="nsmall", bufs=1) as nsmall,
            tc.tile_pool(name="ntmp", bufs=1) as ntmp,
            tc.tile_pool(name="cps", bufs=3, space="PSUM") as cps,
            tc.tile_pool(name="opsum", bufs=2, space="PSUM") as opsum,
            tc.tile_pool(name="spsum", bufs=2, space="PSUM") as spsum,
        ):
            # ======== consts / conv weights ========
            ZR = wpool.tile([P, 2, Wp], f32)
            nc.gpsimd.memset(ZR[:], 0.0)
            wcr = wpool.tile([P, 4, 9, P], f32r)
            nc.sync.dma_start(wcr[:], wconv_d[:].rearrange("l t ci co -> ci l t co"))
            bc = wpool.tile([P, 4], f32)
            nc.sync.dma_start(bc[:], bconv_d[:].rearrange("l c -> c l"))
            w1r = wpool.tile([P, 5], f32r)
            nc.sync.dma_start(w1r[:], w1x1_d[:])

            FR = [conv.tile([P, R + 4, Wp], f32r, name=f"fr{i}") for i in range(2)]
            X1 = {b: conv.tile([P, R + 2, Wp], f32r, name=f"x1b{b}") for b in (0, 1)}
            X2 = {b: conv.tile([P, R, W], f32r, name=f"x2b{b}") for b in (0, 1)}
            for tl in FR:
                nc.gpsimd.memset(tl[:, :, 0:1].bitcast(f32), 0.0)
                nc.gpsimd.memset(tl[:, :, Wp - 1 : Wp].bitcast(f32), 0.0)
            for b in (0, 1):
                nc.gpsimd.memset(X1[b][:, :, 0:1].bitcast(f32), 0.0)
                nc.gpsimd.memset(X1[b][:, :, Wp - 1 : Wp].bitcast(f32), 0.0)

            # ======== cls branch ========
            fcls = clsp.tile([P, 2, 16, 16], f32)
            nc.sync.dma_start(fcls[:], fclsT_d[:].rearrange("(k ci) a b -> ci k a b", ci=P))
            wcl = clsp.tile([P, 9, 2, P], f32)
            nc.sync.dma_start(wcl[:], wcls_d[:].rearrange("t k ci co -> ci t k co"))
            bcl = clsp.tile([P, 1], f32)
            nc.sync.dma_start(bcl[:], bcls_d[:])
            fw1 = clsp.tile([P, P], f32)
            nc.sync.dma_start(fw1[:], fw1_d[:])
            fb1 = clsp.tile([P, 1], f32)
            nc.sync.dma_start(fb1[:], fb1_d[:])
            fw2 = clsp.tile([P, P], f32)
            nc.sync.dma_start(fw2[:], fw2_d[:])
            fb2 = clsp.tile([P, 1], f32)
            nc.sync.dma_start(fb2[:], fb2_d[:])
            ow = clsp.tile([P, 3], f32)
            nc.sync.dma_start(ow[:], ow_d[:])
            ob = clsp.tile([3, 1], f32)
            nc.sync.dma_start(ob[:], ob_d[:])
            i3 = clsp.tile([3, 3], f32)
            nc.sync.dma_start(i3[:], i3_d[:])

            cls_ps1 = spsum.tile([P, 25], f32, name="sps1", tag="sps")
            for t in range(9):
                dy, dx = t // 3, t % 3
                for k in range(2):
                    nc.tensor.matmul(
                        cls_ps1[:], wcl[:, t, k, :],
                        fcls[:, k, dy : dy + 13 : 3, dx : dx + 13 : 3],
                        start=(t == 0 and k == 0), stop=(t == 8 and k == 1),
                    )
            crelu = clsp.tile([P, 25], f32)
            nc.scalar.activation(crelu[:], cls_ps1[:], Act.Relu, bias=bcl[:])
            gap = clsp.tile([P, 1], f32)
            nc.vector.tensor_reduce(out=gap[:], in_=crelu[:], axis=AX.X, op=Alu.add)
            nc.vector.tensor_scalar(out=gap[:], in0=gap[:], scalar1=1.0 / 25.0, scalar2=None, op0=Alu.mult)
            cls_ps2 = spsum.tile([P, 1], f32, name="sps2", tag="sps")
            nc.tensor.matmul(cls_ps2[:], fw1[:], gap[:], start=True, stop=True)
            h1 = clsp.tile([P, 1], f32)
            nc.scalar.activation(h1[:], cls_ps2[:], Act.Relu, bias=fb1[:])
            cls_ps3 = spsum.tile([P, 1], f32, name="sps3", tag="sps")
            nc.tensor.matmul(cls_ps3[:], fw2[:], h1[:], start=True, stop=True)
            h2 = clsp.tile([P, 1], f32)
            nc.scalar.activation(h2[:], cls_ps3[:], Act.Relu, bias=fb2[:])
            cls_ps4 = spsum.tile([3, 1], f32, name="sps4", tag="sps")
            nc.tensor.matmul(cls_ps4[:], ow[:], h2[:], start=True, stop=True)
            logit = clsp.tile([3, 1], f32)
            nc.vector.tensor_tensor(out=logit[:], in0=cls_ps4[:], in1=ob[:], op=Alu.add)
            cls_ps5 = spsum.tile([1, 3], f32, name="sps5", tag="sps")
            nc.tensor.matmul(cls_ps5[:], logit[:], i3[:], start=True, stop=True)
            lrow = clsp.tile([1, 3], f32)
            nc.vector.tensor_copy(lrow[:], cls_ps5[:])
            lmax = clsp.tile([1, 1], f32)
            nc.vector.tensor_reduce(out=lmax[:], in_=lrow[:], axis=AX.X, op=Alu.max)
            esub = clsp.tile([1, 3], f32)
            nc.vector.tensor_scalar(out=esub[:], in0=lrow[:], scalar1=lmax[:], scalar2=None, op0=Alu.subtract)
            eexp = clsp.tile([1, 3], f32)
            nc.scalar.activation(eexp[:], esub[:], Act.Exp)
            esum = clsp.tile([1, 1], f32)
            nc.vector.tensor_reduce(out=esum[:], in_=eexp[:], axis=AX.X, op=Alu.add)
            einv = clsp.tile([1, 1], f32)
            nc.vector.reciprocal(einv[:], esum[:])
            prob = clsp.tile([1, 3], f32)
            nc.vector.tensor_scalar(out=prob[:], in0=eexp[:], scalar1=einv[:], scalar2=None, op0=Alu.mult)
            nc.scalar.dma_start(opred_d[:], prob[:])

            # ======== NMS part 1: loads, rank, scatter, reload, M01 ========
            s_row = nsmall.tile([1, NBOX], f32)
            nc.sync.dma_start(s_row[:], scores_d[:])
            s_b = nbig.tile([P, NBOX], f32, tag="s_b")
            nc.gpsimd.partition_broadcast(s_b[:], s_row[:])
            s_c = nsmall.tile([P, NCH], f32)
            nc.sync.dma_start(s_c[:], scores_d[0, :].rearrange("(c p) -> p c", p=P))
            tri = nsmall.tile([P, P], f32)
            nc.sync.dma_start(tri[:], tri_d[:])
            tri_m = nsmall.tile([P, P], M01DT)
            nc.vector.tensor_copy(tri_m[:], tri[:])
            trilo = nsmall.tile([P, P], f32)
            nc.sync.dma_start(trilo[:], trilo_d[:])
            trilo_bf = nsmall.tile([P, P], bf16)
            nc.vector.tensor_copy(trilo_bf[:], trilo[:])
            b_c = nsmall.tile([P, NCH, 4], f32)
            nc.sync.dma_start(b_c[:], boxes_d[:].rearrange("(c p) k -> p c k", p=P))

            dy_c = nsmall.tile([P, NCH], f32)
            dx_c = nsmall.tile([P, NCH], f32)
            area_c = nsmall.tile([P, NCH], f32)
            nc.vector.tensor_tensor(out=dy_c[:], in0=b_c[:, :, 2], in1=b_c[:, :, 0], op=Alu.subtract)
            nc.vector.tensor_tensor(out=dx_c[:], in0=b_c[:, :, 3], in1=b_c[:, :, 1], op=Alu.subtract)
            nc.vector.tensor_tensor(out=area_c[:], in0=dy_c[:], in1=dx_c[:], op=Alu.mult)

            gt_c = nsmall.tile([P, NCH], f32)
            tie1_c = nsmall.tile([P, NCH], f32)
            tie2_c = nsmall.tile([P, NCH], f32)
            rank_f = nsmall.tile([P, NCH], f32)
            nc.vector.memset(tie1_c[:], 0.0)
            for c in range(NCH):
                t0 = ntmp.tile([P, NBOX], bf16, name=f"t0_{c}", tag="u0")
                nc.vector.tensor_scalar(
                    out=t0[:], in0=s_b[:], scalar1=s_c[:, c : c + 1], scalar2=None,
                    op0=Alu.is_gt, op1=Alu.add, accum_out=gt_c[:, c : c + 1],
                )
                Wc = (c + 1) * P
                eqt = ntmp.tile([P, NBOX], bf16, name=f"eqt_{c}", tag="u1")
                nc.vector.tensor_scalar(
                    out=eqt[:, :Wc], in0=s_b[:, :Wc], scalar1=s_c[:, c : c + 1],
                    scalar2=None, op0=Alu.is_equal,
                )
                if c > 0:
                    nc.vector.tensor_reduce(
                        out=tie1_c[:, c : c + 1], in_=eqt[:, : c * P], axis=AX.X, op=Alu.add
                    )
                eqw = ntmp.tile([P, P], bf16, name=f"eqw_{c}", tag="eqw")
                nc.vector.tensor_tensor(out=eqw[:], in0=eqt[:, c * P : Wc], in1=trilo_bf[:], op=Alu.mult)
                nc.vector.tensor_reduce(
                    out=tie2_c[:, c : c + 1], in_=eqw[:], axis=AX.X, op=Alu.add
                )
            nc.vector.tensor_tensor(out=rank_f[:], in0=gt_c[:], in1=tie1_c[:], op=Alu.add)
            nc.vector.tensor_tensor(out=rank_f[:], in0=rank_f[:], in1=tie2_c[:], op=Alu.add)
            rank_i = nsmall.tile([P, NCH], i32)
            nc.vector.tensor_copy(rank_i[:], rank_f[:])

            for c in range(NCH):
                st = ntmp.tile([P, 6], f32, name=f"st_{c}", tag="st", bufs=2)
                nc.vector.tensor_copy(st[:, 0:4], b_c[:, c, :])
                nc.vector.tensor_copy(st[:, 4:5], s_c[:, c : c + 1])
                nc.vector.tensor_copy(st[:, 5:6], area_c[:, c : c + 1])
                nc.gpsimd.indirect_dma_start(
                    out=scratch_d[:],
                    out_offset=bass.IndirectOffsetOnAxis(ap=rank_i[:, c : c + 1], axis=0),
                    in_=st[:],
                    in_offset=None,
                )

            y1b = nbig.tile([P, NBOX], f32, tag="y1b")
            x1b = nbig.tile([P, NBOX], f32, tag="x1b")
            y2b = nbig.tile([P, NBOX], f32, tag="y2b")
            x2b = nbig.tile([P, NBOX], f32, tag="x2b")
            arb = nbig.tile([P, NBOX], f32, tag="s_b")  # reuse s_b slot (rank done)
            for t, k in ((y1b, 0), (x1b, 1), (y2b, 2), (x2b, 3), (arb, 5)):
                rr = ntmp.tile([1, NBOX], f32, name=f"rr{k}", tag="rr", bufs=1)
                nc.gpsimd.dma_start(rr[:], scratch_d[:, k].rearrange("(o n) -> o n", o=1))
                nc.gpsimd.partition_broadcast(t[:], rr[:])
            sc_s = nsmall.tile([P, NCH, 6], f32)
            nc.gpsimd.dma_start(sc_s[:], scratch_d[:].rearrange("(c p) k -> p c k", p=P))

            valid_c = nsmall.tile([P, NCH], f32)
            nc.vector.tensor_scalar(
                out=valid_c[:], in0=sc_s[:, :, 4], scalar1=SCORE_THR, scalar2=None, op0=Alu.is_ge
            )

            m01 = []
            for a in range(NCH):
                Wa = NBOX - a * P
                m = nbig.tile([P, Wa], M01DT, name=f"m01_{a}", tag=f"m01_{a}")
                m01.append(m)
                ja = a * P
                u0 = ntmp.tile([P, Wa], f32, name=f"u0_{a}", tag="u0")
                u1 = ntmp.tile([P, Wa], f32, name=f"u1_{a}", tag="u1")
                u2 = ntmp.tile([P, Wa], f32, name=f"u2_{a}", tag="u2")
                nc.vector.tensor_scalar(out=u0[:], in0=y1b[:, ja:], scalar1=sc_s[:, a, 0:1], scalar2=None, op0=Alu.max)
                nc.vector.tensor_scalar(out=u1[:], in0=y2b[:, ja:], scalar1=sc_s[:, a, 2:3], scalar2=None, op0=Alu.min)
                nc.vector.tensor_tensor(out=u0[:], in0=u1[:], in1=u0[:], op=Alu.subtract)
                nc.vector.tensor_scalar(out=u0[:], in0=u0[:], scalar1=0.0, scalar2=None, op0=Alu.max)
                nc.vector.tensor_scalar(out=u1[:], in0=x1b[:, ja:], scalar1=sc_s[:, a, 1:2], scalar2=None, op0=Alu.max)
                nc.vector.tensor_scalar(out=u2[:], in0=x2b[:, ja:], scalar1=sc_s[:, a, 3:4], scalar2=None, op0=Alu.min)
                nc.vector.tensor_tensor(out=u1[:], in0=u2[:], in1=u1[:], op=Alu.subtract)
                nc.vector.tensor_scalar(out=u1[:], in0=u1[:], scalar1=0.0, scalar2=None, op0=Alu.max)
                nc.vector.tensor_tensor(out=u0[:], in0=u0[:], in1=u1[:], op=Alu.mult)
                nc.vector.tensor_scalar(out=u1[:], in0=arb[:, ja:], scalar1=sc_s[:, a, 5:6], scalar2=None, op0=Alu.add)
                nc.vector.tensor_tensor(out=u1[:], in0=u1[:], in1=u0[:], op=Alu.subtract)
                nc.vector.tensor_scalar(out=u1[:], in0=u1[:], scalar1=1e-8, scalar2=0.5, op0=Alu.add, op1=Alu.mult)
                nc.vector.tensor_tensor(out=m[:], in0=u0[:], in1=u1[:], op=Alu.is_gt)
            m01d = []
            for a in range(NCH):
                md = nsmall.tile([P, P], M01DT, name=f"m01d_{a}", tag=f"m01d_{a}")
                m01d.append(md)
                nc.vector.tensor_tensor(out=md[:], in0=m01[a][:, 0:P], in1=tri_m[:], op=Alu.mult)

            # ======== conv branches ========
            npair1 = (R + 2) // 2
            npair2 = R // 2
            for bi in range(NBLK):
                r0 = bi * R
                g0 = r0 - 2
                Fr = FR[bi % 2]
                lo = max(g0, 0)
                hi = min(g0 + R + 4, H)
                soff = lo - g0
                nc.sync.dma_start(Fr[:, soff : soff + hi - lo, 1 : W + 1], featT_d[:, lo:hi, :])
                if bi == 0:
                    nc.scalar.activation(Fr[:, 0:2, :], ZR[:], Act.Copy)
                if bi == NBLK - 1:
                    nc.scalar.activation(Fr[:, R + 2 : R + 4, :], ZR[:], Act.Copy)

                for br in (0, 1):
                    l1, l2 = (0, 1) if br == 0 else (2, 3)
                    for q in range(npair1):
                        pp = cps.tile([P, 2, W], f32, name=f"pp{bi}_{br}_{q}", tag="pp")
                        for t in range(9):
                            dy, dx = t // 3, t % 3
                            nc.tensor.matmul(
                                pp[:], wcr[:, l1, t, :],
                                Fr[:, 2 * q + dy : 2 * q + dy + 2, dx : dx + W],
                                start=(t == 0), stop=(t == 8),
                            )
                        nc.scalar.activation(
                            X1[br][:, 2 * q : 2 * q + 2, 1 : W + 1], pp[:], Act.Relu,
                            bias=bc[:, l1 : l1 + 1],
                        )
                    if bi == 0:
                        nc.scalar.activation(X1[br][:, 0:1, :], ZR[:, 0:1, :], Act.Copy)
                    if bi == NBLK - 1:
                        nc.scalar.activation(X1[br][:, R + 1 : R + 2, :], ZR[:, 0:1, :], Act.Copy)
                    for q in range(npair2):
                        pp = cps.tile([P, 2, W], f32, name=f"pq{bi}_{br}_{q}", tag="pp")
                        for t in range(9):
                            dy, dx = t // 3, t % 3
                            nc.tensor.matmul(
                                pp[:], wcr[:, l2, t, :],
                                X1[br][:, 2 * q + dy : 2 * q + dy + 2, dx : dx + W],
                                start=(t == 0), stop=(t == 8),
                            )
                        nc.scalar.activation(
                            X2[br][:, 2 * q : 2 * q + 2, :], pp[:], Act.Relu,
                            bias=bc[:, l2 : l2 + 1],
                        )
                x2a = X2[0][:].rearrange("p r w -> p (r w)")
                x2bb = X2[1][:].rearrange("p r w -> p (r w)")
                OD = conv.tile([34, R * W], f32, name=f"od{bi}", tag="od", bufs=1)
                for n in range(NCHK):
                    pa = opsum.tile([3, 512], f32, name=f"pa{bi}_{n}", tag="pab")
                    pb = opsum.tile([2, 512], f32, name=f"pb{bi}_{n}", tag="pab")
                    nc.tensor.matmul(pa[:], w1r[:, 0:3], x2a[:, n * 512 : (n + 1) * 512], start=True, stop=True)
                    nc.tensor.matmul(pb[:], w1r[:, 3:5], x2bb[:, n * 512 : (n + 1) * 512], start=True, stop=True)
                    nc.scalar.activation(OD[0:3, n * 512 : (n + 1) * 512], pa[:], Act.Copy)
                    nc.scalar.activation(OD[32:34, n * 512 : (n + 1) * 512], pb[:], Act.Copy)
                nc.scalar.dma_start(oofs_d[:, r0 * W : (r0 + R) * W], OD[0:2, :])
                nc.scalar.dma_start(owt_d[:, r0 * W : (r0 + R) * W], OD[2:3, :])
                nc.scalar.dma_start(osz_d[:, r0 * W : (r0 + R) * W], OD[32:34, :])

            # ======== NMS part 2: chunked greedy suppression + outputs ========
            keeps = []
            keep_f = nsmall.tile([P, NCH], f32)
            for b in range(NCH):
                alive = nsmall.tile([P, 1], f32, name=f"alive_{b}", tag=f"alive_{b}")
                if b == 0:
                    nc.vector.tensor_copy(alive[:], valid_c[:, 0:1])
                else:
                    sup_ps = spsum.tile([P, 1], f32, name=f"cps_{b}", tag="sps")
                    for a in range(b):
                        nc.tensor.matmul(
                            sup_ps[:], m01[a][:, (b - a) * P : (b - a + 1) * P], keeps[a][:],
                            start=(a == 0), stop=(a == b - 1),
                        )
                    nc.vector.tensor_scalar(
                        out=alive[:], in0=sup_ps[:], scalar1=0.0, scalar2=valid_c[:, b : b + 1],
                        op0=Alu.is_equal, op1=Alu.mult,
                    )
                kb = nsmall.tile([P, 1], M01DT, name=f"keep_{b}", tag=f"keep_{b}")
                keeps.append(kb)
                nc.vector.tensor_copy(kb[:], alive[:])
                for t in range(NITER):
                    ips = spsum.tile([P, 1], f32, name=f"ips_{b}_{t}", tag="sps")
                    nc.tensor.matmul(ips[:], m01d[b][:], kb[:], start=True, stop=True)
                    nc.vector.tensor_scalar(
                        out=kb[:], in0=ips[:], scalar1=0.0, scalar2=alive[:],
                        op0=Alu.is_equal, op1=Alu.mult,
                    )
                nc.vector.tensor_copy(keep_f[:, b : b + 1], kb[:])

            for c in range(NCH):
                obx = ntmp.tile([P, 4], f32, name=f"obx_{c}", tag="obx", bufs=2)
                osc = ntmp.tile([P, 1], f32, name=f"osc_{c}", tag="osc", bufs=2)
                nc.vector.tensor_scalar(
                    out=obx[:], in0=sc_s[:, c, 0:4], scalar1=keep_f[:, c : c + 1],
                    scalar2=None, op0=Alu.mult,
                )
                nc.vector.tensor_scalar(
                    out=osc[:], in0=sc_s[:, c, 4:5], scalar1=keep_f[:, c : c + 1],
                    scalar2=None, op0=Alu.mult,
                )
                nc.gpsimd.dma_start(onmsb_d[:].rearrange("(c p) k -> p k c", p=P)[:, :, c], obx[:])
                nc.gpsimd.dma_start(onmss_d[0, :].rearrange("(c p) -> p c", p=P)[:, c : c + 1], osc[:])

    nc.compile()
    return nc


def kernel(feat_reg, feat_cls, cls, boxes, scores, params):
    global _BUILT
    trace = os.environ.get("KERNEL_TRACE", "0") == "1"
    if trace:
        _install_prof_shim()
    from concourse.bass_utils import run_bass_kernel_spmd

    feat_reg = np.asarray(feat_reg, dtype=np.float32)
    feat_cls = np.asarray(feat_cls, dtype=np.float32)
    cls = np.asarray(cls, dtype=np.int32)
    boxes = np.asarray(boxes, dtype=np.float32)
    scores = np.asarray(scores, dtype=np.float32)
    pr = {k: np.asarray(v, dtype=np.float32) for k, v in params.items()}

    if _BUILT is None:
        _BUILT = _build()
    nc = _BUILT

    B = feat_reg.shape[0]
    wconv = np.stack(
        [pr["reg_w1"], pr["reg_w2"], pr["sz_w1"], pr["sz_w2"]]
    ).reshape(4, 9, P, P).astype(np.float32)
    bconv = np.stack(
        [pr["reg_b1"], pr["reg_b2"], pr["sz_b1"], pr["sz_b2"]]
    ).astype(np.float32)
    wcls = pr["cls_cw"].reshape(9, 256, P).reshape(9, 2, P, P).astype(np.float32)
    tri = (np.arange(P)[:, None] < np.arange(P)[None, :]).astype(np.float32)

    in_maps = []
    for b in range(B):
        c = int(cls[b])
        w1x1 = np.concatenate(
            [pr["ofs_w"][:, 2 * c : 2 * c + 2], pr["wt_w"][:, c : c + 1],
             pr["size_w"][:, 2 * c : 2 * c + 2]], axis=1
        ).astype(np.float32)
        in_maps.append({
            "featT": np.ascontiguousarray(feat_reg[b].transpose(2, 0, 1)),
            "fclsT": np.ascontiguousarray(feat_cls[b].transpose(2, 0, 1)),
            "wconv": wconv, "bconv": bconv, "w1x1": w1x1,
            "wcls": wcls, "bcls": pr["cls_cb"].reshape(P, 1),
            "fw1": pr["cls_fw1"], "fb1": pr["cls_fb1"].reshape(P, 1),
            "fw2": pr["cls_fw2"], "fb2": pr["cls_fb2"].reshape(P, 1),
            "ow": pr["cls_ow"], "ob": pr["cls_ob"].reshape(3, 1),
            "i3": np.eye(3, dtype=np.float32),
            "boxes": boxes[b], "scores": scores[b].reshape(1, NBOX),
            "tri": tri, "trilo": np.ascontiguousarray(tri.T),
        })

    res = run_bass_kernel_spmd(nc, in_maps, list(range(8)), trace=trace)
    if trace and res.exec_time_ns is not None:
        print(f"HW exec time: {res.exec_time_ns} ns")

    offsets_sel = np.stack(
        [res.results[b]["oofs"].reshape(2, H, W).transpose(1, 2, 0) for b in range(B)]
    )
    sizes_sel = np.stack(
        [res.results[b]["osz"].reshape(2, H, W).transpose(1, 2, 0) for b in range(B)]
    )
    weights_sel = np.stack(
        [res.results[b]["owt"].reshape(1, H, W).transpose(1, 2, 0) for b in range(B)]
    )
    pred_cls = np.stack([res.results[b]["opred"][0] for b in range(B)])
    nms_boxes = np.stack([res.results[b]["onmsb"] for b in range(B)])
    nms_scores = np.stack([res.results[b]["onmss"][0] for b in range(B)])
    return offsets_sel, sizes_sel, weights_sel, pred_cls, nms_boxes, nms_scores


# revision 3
# speedup vs baseline: 1.0788x; 1.0788x over previous
"""Trainium2 Bass kernel for nn_DetModel (conv heads + per-sample NMS).

Data-parallel over batch: 8 samples -> 8 NeuronCores. Each core runs
two conv3x3+conv3x3+1x1 branches (f32r matmuls), the small cls branch,
and an exact greedy NMS (rank -> scatter-sort -> chunked IoU ->
fixed-point suppression).
"""
import os
import numpy as np

P = 128
H = W = 256
R = 8
NBLK = H // R
Wp = W + 2
NBOX = 2048
NCH = NBOX // P
NITER = 6
SCORE_THR = 0.05

_BUILT = None


def _install_prof_shim():
    import sys
    import types

    if "antenv.axon_hooks" not in sys.modules:
        from trn_agent_boot.trn_boot import _ntff_profile_via_ctypes

        hook = _ntff_profile_via_ctypes("/opt/axon/libaxon_pjrt.so")
        mod = types.ModuleType("antenv.axon_hooks")
        mod._hook = hook
        mod.get_axon_ntff_profile_hook = lambda: mod._hook
        mod.set_axon_ntff_profile_hook = lambda h: setattr(mod, "_hook", h)
        sys.modules["antenv.axon_hooks"] = mod
    import concourse.bass_utils as bu

    bu.upload_artifacts = lambda tmpdir: f"local:{tmpdir}"


def _build():
    import concourse.bass as bass
    import concourse.bacc as bacc
    import concourse.mybir as mybir
    import concourse.tile as tile

    f32 = mybir.dt.float32
    f32r = mybir.dt.float32r
    bf16 = mybir.dt.bfloat16
    i32 = mybir.dt.int32
    Alu = mybir.AluOpType
    AX = mybir.AxisListType
    Act = mybir.ActivationFunctionType
    M01DT = mybir.dt.float8e4

    nc = bacc.Bacc("TRN2", target_bir_lowering=False, debug=False, num_devices=8)

    featT_d = nc.dram_tensor("featT", [P, H, W], f32r, kind="ExternalInput")
    fclsT_d = nc.dram_tensor("fclsT", [256, 16, 16], f32, kind="ExternalInput")
    wconv_d = nc.dram_tensor("wconv", [4, 9, P, P], f32r, kind="ExternalInput")
    bconv_d = nc.dram_tensor("bconv", [4, P], f32, kind="ExternalInput")
    w1x1_d = nc.dram_tensor("w1x1", [P, 5], f32r, kind="ExternalInput")
    wcls_d = nc.dram_tensor("wcls", [9, 2, P, P], f32, kind="ExternalInput")
    bcls_d = nc.dram_tensor("bcls", [P, 1], f32, kind="ExternalInput")
    fw1_d = nc.dram_tensor("fw1", [P, P], f32, kind="ExternalInput")
    fb1_d = nc.dram_tensor("fb1", [P, 1], f32, kind="ExternalInput")
    fw2_d = nc.dram_tensor("fw2", [P, P], f32, kind="ExternalInput")
    fb2_d = nc.dram_tensor("fb2", [P, 1], f32, kind="ExternalInput")
    ow_d = nc.dram_tensor("ow", [P, 3], f32, kind="ExternalInput")
    ob_d = nc.dram_tensor("ob", [3, 1], f32, kind="ExternalInput")
    i3_d = nc.dram_tensor("i3", [3, 3], f32, kind="ExternalInput")
    boxes_d = nc.dram_tensor("boxes", [NBOX, 4], f32, kind="ExternalInput")
    scores_d = nc.dram_tensor("scores", [1, NBOX], f32, kind="ExternalInput")
    tri_d = nc.dram_tensor("tri", [P, P], f32, kind="ExternalInput")
    trilo_d = nc.dram_tensor("trilo", [P, P], f32, kind="ExternalInput")

    scratch_d = nc.dram_tensor("scratch", [NBOX, 6], f32)

    oofs_d = nc.dram_tensor("oofs", [2, H * W], f32, kind="ExternalOutput")
    osz_d = nc.dram_tensor("osz", [2, H * W], f32, kind="ExternalOutput")
    owt_d = nc.dram_tensor("owt", [1, H * W], f32, kind="ExternalOutput")
    opred_d = nc.dram_tensor("opred", [1, 3], f32, kind="ExternalOutput")
    onmsb_d = nc.dram_tensor("onmsb", [NBOX, 4], f32, kind="ExternalOutput")
    onmss_d = nc.dram_tensor("onmss", [1, NBOX], f32, kind="ExternalOutput")

    NCHK = R * W // 512

    with tile.TileContext(nc) as tc:
        with (
            tc.tile_pool(name="wpool", bufs=1) as wpool,
            tc.tile_pool(name="conv", bufs=1) as conv,
            tc.tile_pool(name="cls", bufs=1) as clsp,
            tc.tile_pool(name="nbig", bufs=1) as nbig,
            tc.tile_pool(name="nsmall", bufs=1) as nsmall,
            tc.tile_pool(name="ntmp", bufs=1) as ntmp,
            tc.tile_pool(name="cps", bufs=3, space="PSUM") as cps,
            tc.tile_pool(name="opsum", bufs=2, space="PSUM") as opsum,
            tc.tile_pool(name="spsum", bufs=2, space="PSUM") as spsum,
        ):
            # ======== consts / conv weights ========
            ZR = wpool.tile([P, 2, Wp], f32)
            nc.gpsimd.memset(ZR[:], 0.0)
            wcr = wpool.tile([P, 4, 9, P], f32r)
            nc.sync.dma_start(wcr[:], wconv_d[:].rearrange("l t ci co -> ci l t co"))
            bc = wpool.tile([P, 4], f32)
            nc.sync.dma_start(bc[:], bconv_d[:].rearrange("l c -> c l"))
            w1r = wpool.tile([P, 5], f32r)
            nc.sync.dma_start(w1r[:], w1x1_d[:])

            FR = [conv.tile([P, R + 4, Wp], f32r, name=f"fr{i}") for i in range(2)]
            X1 = {b: conv.tile([P, R + 2, Wp], f32r, name=f"x1b{b}") for b in (0, 1)}
            X2 = {b: conv.tile([P, R, W], f32r, name=f"x2b{b}") for b in (0, 1)}
            for tl in FR:
                nc.gpsimd.memset(tl[:, :, 0:1].bitcast(f32), 0.0)
                nc.gpsimd.memset(tl[:, :, Wp - 1 : Wp].bitcast(f32), 0.0)
            for b in (0, 1):
                nc.gpsimd.memset(X1[b][:, :, 0:1].bitcast(f32), 0.0)
                nc.gpsimd.memset(X1[b][:, :, Wp - 1 : Wp].bitcast(f32), 0.0)

            # ======== cls branch ========
            fcls = clsp.tile([P, 2, 16, 16], f32)
            nc.sync.dma_start(fcls[:], fclsT_d[:].rearrange("(k ci) a b -> ci k a b", ci=P))
            wcl = clsp.tile([P, 9, 2, P], f32)
            nc.sync.dma_start(wcl[:], wcls_d[:].rearrange("t k ci co -> ci t k co"))
            bcl = clsp.tile([P, 1], f32)
            nc.sync.dma_start(bcl[:], bcls_d[:])
            fw1 = clsp.tile([P, P], f32)
            nc.sync.dma_start(fw1[:], fw1_d[:])
            fb1 = clsp.tile([P, 1], f32)
            nc.sync.dma_start(fb1[:], fb1_d[:])
            fw2 = clsp.tile([P, P], f32)
            nc.sync.dma_start(fw2[:], fw2_d[:])
            fb2 = clsp.tile([P, 1], f32)
            nc.sync.dma_start(fb2[:], fb2_d[:])
            ow = clsp.tile([P, 3], f32)
            nc.sync.dma_start(ow[:], ow_d[:])
            ob = clsp.tile([3, 1], f32)
            nc.sync.dma_start(ob[:], ob_d[:])
            i3 = clsp.tile([3, 3], f32)
            nc.sync.dma_start(i3[:], i3_d[:])

            cls_ps1 = spsum.tile([P, 25], f32, name="sps1", tag="sps")
            for t in range(9):
                dy, dx = t // 3, t % 3
                for k in range(2):
                    nc.tensor.matmul(
                        cls_ps1[:], wcl[:, t, k, :],
                        fcls[:, k, dy : dy + 13 : 3, dx : dx + 13 : 3],
                        start=(t == 0 and k == 0), stop=(t == 8 and k == 1),
                    )
            crelu = clsp.tile([P, 25], f32)
            nc.scalar.activation(crelu[:], cls_ps1[:], Act.Relu, bias=bcl[:])
            gap = clsp.tile([P, 1], f32)
            nc.vector.tensor_reduce(out=gap[:], in_=crelu[:], axis=AX.X, op=Alu.add)
            nc.vector.tensor_scalar(out=gap[:], in0=gap[:], scalar1=1.0 / 25.0, scalar2=None, op0=Alu.mult)
            cls_ps2 = spsum.tile([P, 1], f32, name="sps2", tag="sps")
            nc.tensor.matmul(cls_ps2[:], fw1[:], gap[:], start=True, stop=True)
            h1 = clsp.tile([P, 1], f32)
            nc.scalar.activation(h1[:], cls_ps2[:], Act.Relu, bias=fb1[:])
            cls_ps3 = spsum.tile([P, 1], f32, name="sps3", tag="sps")
            nc.tensor.matmul(cls_ps3[:], fw2[:], h1[:], start=True, stop=True)
            h2 = clsp.tile([P, 1], f32)
            nc.scalar.activation(h2[:], cls_ps3[:], Act.Relu, bias=fb2[:])
            cls_ps4 = spsum.tile([3, 1], f32, name="sps4", tag="sps")
            nc.tensor.matmul(cls_ps4[:], ow[:], h2[:], start=True, stop=True)
            logit = clsp.tile([3, 1], f32)
            nc.vector.tensor_tensor(out=logit[:], in0=cls_ps4[:], in1=ob[:], op=Alu.add)
            cls_ps5 = spsum.tile([1, 3], f32, name="sps5", tag="sps")
            nc.tensor.matmul(cls_ps5[:], logit[:], i3[:], start=True, stop=True)
            lrow = clsp.tile([1, 3], f32)
            nc.vector.tensor_copy(lrow[:], cls_ps5[:])
            lmax = clsp.tile([1, 1], f32)
            nc.vector.tensor_reduce(out=lmax[:], in_=lrow[:], axis=AX.X, op=Alu.max)
            esub = clsp.tile([1, 3], f32)
            nc.vector.tensor_scalar(out=esub[:], in0=lrow[:], scalar1=lmax[:], scalar2=None, op0=Alu.subtract)
            eexp = clsp.tile([1, 3], f32)
            nc.scalar.activation(eexp[:], esub[:], Act.Exp)
            esum = clsp.tile([1, 1], f32)
            nc.vector.tensor_reduce(out=esum[:], in_=eexp[:], axis=AX.X, op=Alu.add)
            einv = clsp.tile([1, 1], f32)
            nc.vector.reciprocal(einv[:], esum[:])
            prob = clsp.tile([1, 3], f32)
            nc.vector.tensor_scalar(out=prob[:], in0=eexp[:], scalar1=einv[:], scalar2=None, op0=Alu.mult)
            nc.scalar.dma_start(opred_d[:], prob[:])

            # ======== NMS part 1: loads, rank, scatter, reload, M01 ========
            s_row = nsmall.tile([1, NBOX], f32)
            nc.sync.dma_start(s_row[:], scores_d[:])
            s_b = nbig.tile([P, NBOX], f32, tag="s_b")
            nc.gpsimd.partition_broadcast(s_b[:], s_row[:])
            s_c = nsmall.tile([P, NCH], f32)
            nc.sync.dma_start(s_c[:], scores_d[0, :].rearrange("(c p) -> p c", p=P))
            tri = nsmall.tile([P, P], f32)
            nc.sync.dma_start(tri[:], tri_d[:])
            tri_m = nsmall.tile([P, P], M01DT)
            nc.vector.tensor_copy(tri_m[:], tri[:])
            trilo = nsmall.tile([P, P], f32)
            nc.sync.dma_start(trilo[:], trilo_d[:])
            trilo_bf = nsmall.tile([P, P], bf16)
            nc.vector.tensor_copy(trilo_bf[:], trilo[:])
            b_c = nsmall.tile([P, NCH, 4], f32)
            nc.sync.dma_start(b_c[:], boxes_d[:].rearrange("(c p) k -> p c k", p=P))

            dy_c = nsmall.tile([P, NCH], f32)
            dx_c = nsmall.tile([P, NCH], f32)
            area_c = nsmall.tile([P, NCH], f32)
            nc.vector.tensor_tensor(out=dy_c[:], in0=b_c[:, :, 2], in1=b_c[:, :, 0], op=Alu.subtract)
            nc.vector.tensor_tensor(out=dx_c[:], in0=b_c[:, :, 3], in1=b_c[:, :, 1], op=Alu.subtract)
            nc.vector.tensor_tensor(out=area_c[:], in0=dy_c[:], in1=dx_c[:], op=Alu.mult)

            gt_c = nsmall.tile([P, NCH], f32)
            tie1_c = nsmall.tile([P, NCH], f32)
            tie2_c = nsmall.tile([P, NCH], f32)
            rank_f = nsmall.tile([P, NCH], f32)
            nc.vector.memset(tie1_c[:], 0.0)
            for c in range(NCH):
                t0 = ntmp.tile([P, NBOX], bf16, name=f"t0_{c}", tag="u0")
                nc.vector.tensor_scalar(
                    out=t0[:], in0=s_b[:], scalar1=s_c[:, c : c + 1], scalar2=None,
                    op0=Alu.is_gt, op1=Alu.add, accum_out=gt_c[:, c : c + 1],
                )
                Wc = (c + 1) * P
                eqt = ntmp.tile([P, NBOX], bf16, name=f"eqt_{c}", tag="u1")
                nc.vector.tensor_scalar(
                    out=eqt[:, :Wc], in0=s_b[:, :Wc], scalar1=s_c[:, c : c + 1],
                    scalar2=None, op0=Alu.is_equal,
                )
                if c > 0:
                    nc.vector.tensor_reduce(
                        out=tie1_c[:, c : c + 1], in_=eqt[:, : c * P], axis=AX.X, op=Alu.add
                    )
                eqw = ntmp.tile([P, P], bf16, name=f"eqw_{c}", tag="eqw")
                nc.vector.tensor_tensor(out=eqw[:], in0=eqt[:, c * P : Wc], in1=trilo_bf[:], op=Alu.mult)
                nc.vector.tensor_reduce(
                    out=tie2_c[:, c : c + 1], in_=eqw[:], axis=AX.X, op=Alu.add
                )
            nc.vector.tensor_tensor(out=rank_f[:], in0=gt_c[:], in1=tie1_c[:], op=Alu.add)
            nc.vector.tensor_tensor(out=rank_f[:], in0=rank_f[:], in1=tie2_c[:], op=Alu.add)
            rank_i = nsmall.tile([P, NCH], i32)
            nc.vector.tensor_copy(rank_i[:], rank_f[:])

            for c in range(NCH):
                st = ntmp.tile([P, 6], f32, name=f"st_{c}", tag="st", bufs=2)
                nc.vector.tensor_copy(st[:, 0:4], b_c[:, c, :])
                nc.vector.tensor_copy(st[:, 4:5], s_c[:, c : c + 1])
                nc.vector.tensor_copy(st[:, 5:6], area_c[:, c : c + 1])
                nc.gpsimd.indirect_dma_start(
                    out=scratch_d[:],
                    out_offset=bass.IndirectOffsetOnAxis(ap=rank_i[:, c : c + 1], axis=0),
                    in_=st[:],
                    in_offset=None,
                )

            y1b = nbig.tile([P, NBOX], f32, tag="y1b")
            x1b = nbig.tile([P, NBOX], f32, tag="x1b")
            y2b = nbig.tile([P, NBOX], f32, tag="y2b")
            x2b = nbig.tile([P, NBOX], f32, tag="x2b")
            arb = nbig.tile([P, NBOX], f32, tag="s_b")  # reuse s_b slot (rank done)
            for t, k in ((y1b, 0), (x1b, 1), (y2b, 2), (x2b, 3), (arb, 5)):
                rr = ntmp.tile([1, NBOX], f32, name=f"rr{k}", tag="rr", bufs=1)
                nc.gpsimd.dma_start(rr[:], scratch_d[:, k].rearrange("(o n) -> o n", o=1))
                nc.gpsimd.partition_broadcast(t[:], rr[:])
            sc_s = nsmall.tile([P, NCH, 6], f32)
            nc.gpsimd.dma_start(sc_s[:], scratch_d[:].rearrange("(c p) k -> p c k", p=P))

            valid_c = nsmall.tile([P, NCH], f32)
            nc.vector.tensor_scalar(
                out=valid_c[:], in0=sc_s[:, :, 4], scalar1=SCORE_THR, scalar2=None, op0=Alu.is_ge
            )

            m01 = []
            for a in range(NCH):
                Wa = NBOX - a * P
                m = nbig.tile([P, Wa], M01DT, name=f"m01_{a}", tag=f"m01_{a}")
                m01.append(m)
                ja = a * P
                u0 = ntmp.tile([P, Wa], f32, name=f"u0_{a}", tag="u0")
                u1 = ntmp.tile([P, Wa], f32, name=f"u1_{a}", tag="u1")
                u2 = ntmp.tile([P, Wa], f32, name=f"u2_{a}", tag="u2")
                nc.vector.tensor_scalar(out=u0[:], in0=y1b[:, ja:], scalar1=sc_s[:, a, 0:1], scalar2=None, op0=Alu.max)
                nc.vector.tensor_scalar(out=u1[:], in0=y2b[:, ja:], scalar1=sc_s[:, a, 2:3], scalar2=None, op0=Alu.min)
                nc.vector.tensor_tensor(out=u0[:], in0=u1[:], in1=u0[:], op=Alu.subtract)
                nc.vector.tensor_scalar(out=u0[:], in0=u0[:], scalar1=0.0, scalar2=None, op0=Alu.max)
                nc.vector.tensor_scalar(out=u1[:], in0=x1b[:, ja:], scalar1=sc_s[:, a, 1:2], scalar2=None, op0=Alu.max)
                nc.vector.tensor_scalar(out=u2[:], in0=x2b[:, ja:], scalar1=sc_s[:, a, 3:4], scalar2=None, op0=Alu.min)
                nc.vector.tensor_tensor(out=u1[:], in0=u2[:], in1=u1[:], op=Alu.subtract)
                nc.vector.tensor_scalar(out=u1[:], in0=u1[:], scalar1=0.0, scalar2=None, op0=Alu.max)
                nc.vector.tensor_tensor(out=u0[:], in0=u0[:], in1=u1[:], op=Alu.mult)
                nc.vector.tensor_scalar(out=u1[:], in0=arb[:, ja:], scalar1=sc_s[:, a, 5:6], scalar2=None, op0=Alu.add)
                nc.vector.tensor_tensor(out=u1[:], in0=u1[:], in1=u0[:], op=Alu.subtract)
                nc.vector.tensor_scalar(out=u1[:], in0=u1[:], scalar1=1e-8, scalar2=0.5, op0=Alu.add, op1=Alu.mult)
                nc.vector.tensor_tensor(out=m[:], in0=u0[:], in1=u1[:], op=Alu.is_gt)
            m01d = []
            for a in range(NCH):
                md = nsmall.tile([P, P], M01DT, name=f"m01d_{a}", tag=f"m01d_{a}")
                m01d.append(md)
                nc.vector.tensor_tensor(out=md[:], in0=m01[a][:, 0:P], in1=tri_m[:], op=Alu.mult)

            # ======== conv branches ========
            from concourse.tile import add_dep_helper
            conv_last_mm = []
            npair1 = (R + 2) // 2
            npair2 = R // 2
            for bi in range(NBLK):
                r0 = bi * R
                g0 = r0 - 2
                Fr = FR[bi % 2]
                lo = max(g0, 0)
                hi = min(g0 + R + 4, H)
                soff = lo - g0
                nc.sync.dma_start(Fr[:, soff : soff + hi - lo, 1 : W + 1], featT_d[:, lo:hi, :])
                if bi == 0:
                    nc.scalar.activation(Fr[:, 0:2, :], ZR[:], Act.Copy)
                if bi == NBLK - 1:
                    nc.scalar.activation(Fr[:, R + 2 : R + 4, :], ZR[:], Act.Copy)

                for br in (0, 1):
                    l1, l2 = (0, 1) if br == 0 else (2, 3)
                    for q in range(npair1):
                        pp = cps.tile([P, 2, W], f32, name=f"pp{bi}_{br}_{q}", tag="pp")
                        for t in range(9):
                            dy, dx = t // 3, t % 3
                            nc.tensor.matmul(
                                pp[:], wcr[:, l1, t, :],
                                Fr[:, 2 * q + dy : 2 * q + dy + 2, dx : dx + W],
                                start=(t == 0), stop=(t == 8),
                            )
                        nc.scalar.activation(
                            X1[br][:, 2 * q : 2 * q + 2, 1 : W + 1], pp[:], Act.Relu,
                            bias=bc[:, l1 : l1 + 1],
                        )
                    if bi == 0:
                        nc.scalar.activation(X1[br][:, 0:1, :], ZR[:, 0:1, :], Act.Copy)
                    if bi == NBLK - 1:
                        nc.scalar.activation(X1[br][:, R + 1 : R + 2, :], ZR[:, 0:1, :], Act.Copy)
                    for q in range(npair2):
                        pp = cps.tile([P, 2, W], f32, name=f"pq{bi}_{br}_{q}", tag="pp")
                        for t in range(9):
                            dy, dx = t // 3, t % 3
                            nc.tensor.matmul(
                                pp[:], wcr[:, l2, t, :],
                                X1[br][:, 2 * q + dy : 2 * q + dy + 2, dx : dx + W],
                                start=(t == 0), stop=(t == 8),
                            )
                        nc.scalar.activation(
                            X2[br][:, 2 * q : 2 * q + 2, :], pp[:], Act.Relu,
                            bias=bc[:, l2 : l2 + 1],
                        )
                x2a = X2[0][:].rearrange("p r w -> p (r w)")
                x2bb = X2[1][:].rearrange("p r w -> p (r w)")
                OD = conv.tile([34, R * W], f32, name=f"od{bi}", tag="od", bufs=1)
                for n in range(NCHK):
                    pa = opsum.tile([3, 512], f32, name=f"pa{bi}_{n}", tag="pab")
                    pb = opsum.tile([2, 512], f32, name=f"pb{bi}_{n}", tag="pab")
                    nc.tensor.matmul(pa[:], w1r[:, 0:3], x2a[:, n * 512 : (n + 1) * 512], start=True, stop=True)
                    _mm = nc.tensor.matmul(pb[:], w1r[:, 3:5], x2bb[:, n * 512 : (n + 1) * 512], start=True, stop=True)
                    if n == NCHK - 1:
                        conv_last_mm.append(_mm)
                    nc.scalar.activation(OD[0:3, n * 512 : (n + 1) * 512], pa[:], Act.Copy)
                    nc.scalar.activation(OD[32:34, n * 512 : (n + 1) * 512], pb[:], Act.Copy)
                nc.scalar.dma_start(oofs_d[:, r0 * W : (r0 + R) * W], OD[0:2, :])
                nc.scalar.dma_start(owt_d[:, r0 * W : (r0 + R) * W], OD[2:3, :])
                nc.scalar.dma_start(osz_d[:, r0 * W : (r0 + R) * W], OD[32:34, :])

            # ======== NMS part 2: chunked greedy suppression + outputs ========
            keeps = []
            keep_f = nsmall.tile([P, NCH], f32)
            for b in range(NCH):
                alive = nsmall.tile([P, 1], f32, name=f"alive_{b}", tag=f"alive_{b}")
                if b == 0:
                    nc.vector.tensor_copy(alive[:], valid_c[:, 0:1])
                else:
                    sup_ps = spsum.tile([P, 1], f32, name=f"cps_{b}", tag="sps")
                    for a in range(b):
                        _smm = nc.tensor.matmul(
                            sup_ps[:], m01[a][:, (b - a) * P : (b - a + 1) * P], keeps[a][:],
                            start=(a == 0), stop=(a == b - 1),
                        )
                        if a == 0:
                            add_dep_helper(
                                _smm.ins, conv_last_mm[min(14 + b, NBLK - 2)].ins,
                                sync=False, reason="interleave nms after conv",
                            )
                    nc.vector.tensor_scalar(
                        out=alive[:], in0=sup_ps[:], scalar1=0.0, scalar2=valid_c[:, b : b + 1],
                        op0=Alu.is_equal, op1=Alu.mult,
                    )
                kb = nsmall.tile([P, 1], M01DT, name=f"keep_{b}", tag=f"keep_{b}")
                keeps.append(kb)
                nc.vector.tensor_copy(kb[:], alive[:])
                for t in range(NITER):
                    ips = spsum.tile([P, 1], f32, name=f"ips_{b}_{t}", tag="sps")
                    _imm = nc.tensor.matmul(ips[:], m01d[b][:], kb[:], start=True, stop=True)
                    if b == 0 and t == 0:
                        add_dep_helper(
                            _imm.ins, conv_last_mm[14].ins,
                            sync=False, reason="interleave nms after conv",
                        )
                    nc.vector.tensor_scalar(
                        out=kb[:], in0=ips[:], scalar1=0.0, scalar2=alive[:],
                        op0=Alu.is_equal, op1=Alu.mult,
                    )
                nc.vector.tensor_copy(keep_f[:, b : b + 1], kb[:])

            for c in range(NCH):
                obx = ntmp.tile([P, 4], f32, name=f"obx_{c}", tag="obx", bufs=2)
                osc = ntmp.tile([P, 1], f32, name=f"osc_{c}", tag="osc", bufs=2)
                nc.vector.tensor_scalar(
                    out=obx[:], in0=sc_s[:, c, 0:4], scalar1=keep_f[:, c : c + 1],
                    scalar2=None, op0=Alu.mult,
                )
                nc.vector.tensor_scalar(
                    out=osc[:], in0=sc_s[:, c, 4:5], scalar1=keep_f[:, c : c + 1],
                    scalar2=None, op0=Alu.mult,
                )
                nc.gpsimd.dma_start(onmsb_d[:].rearrange("(c p) k -> p k c", p=P)[:, :, c], obx[:])
                nc.gpsimd.dma_start(onmss_d[0, :].rearrange("(c p) -> p c", p=P)[:, c : c + 1], osc[:])

    nc.compile()
    return nc


def kernel(feat_reg, feat_cls, cls, boxes, scores, params):
    global _BUILT
    trace = os.environ.get("KERNEL_TRACE", "0") == "1"
    if trace:
        _install_prof_shim()
    from concourse.bass_utils import run_bass_kernel_spmd

    feat_reg = np.asarray(feat_reg, dtype=np.float32)
    feat_cls = np.asarray(feat_cls, dtype=np.float32)
    cls = np.asarray(cls, dtype=np.int32)
    boxes = np.asarray(boxes, dtype=np.float32)
    scores = np.asarray(scores, dtype=np.float32)
    pr = {k: np.asarray(v, dtype=np.float32) for k, v in params.items()}

    if _BUILT is None:
        _BUILT = _build()
    nc = _BUILT

    B = feat_reg.shape[0]
    wconv = np.stack(
        [pr["reg_w1"], pr["reg_w2"], pr["sz_w1"], pr["sz_w2"]]
    ).reshape(4, 9, P, P).astype(np.float32)
    bconv = np.stack(
        [pr["reg_b1"], pr["reg_b2"], pr["sz_b1"], pr["sz_b2"]]
    ).astype(np.float32)
    wcls = pr["cls_cw"].reshape(9, 256, P).reshape(9, 2, P, P).astype(np.float32)
    tri = (np.arange(P)[:, None] < np.arange(P)[None, :]).astype(np.float32)

    in_maps = []
    for b in range(B):
        c = int(cls[b])
        w1x1 = np.concatenate(
            [pr["ofs_w"][:, 2 * c : 2 * c + 2], pr["wt_w"][:, c : c + 1],
             pr["size_w"][:, 2 * c : 2 * c + 2]], axis=1
        ).astype(np.float32)
        in_maps.append({
            "featT": np.ascontiguousarray(feat_reg[b].transpose(2, 0, 1)),
            "fclsT": np.ascontiguousarray(feat_cls[b].transpose(2, 0, 1)),
            "wconv": wconv, "bconv": bconv, "w1x1": w1x1,
            "wcls": wcls, "bcls": pr["cls_cb"].reshape(P, 1),
            "fw1": pr["cls_fw1"], "fb1": pr["cls_fb1"].reshape(P, 1),
            "fw2": pr["cls_fw2"], "fb2": pr["cls_fb2"].reshape(P, 1),
            "ow": pr["cls_ow"], "ob": pr["cls_ob"].reshape(3, 1),
            "i3": np.eye(3, dtype=np.float32),
            "boxes": boxes[b], "scores": scores[b].reshape(1, NBOX),
            "tri": tri, "trilo": np.ascontiguousarray(tri.T),
        })

    res = run_bass_kernel_spmd(nc, in_maps, list(range(8)), trace=trace)
    if trace and res.exec_time_ns is not None:
        print(f"HW exec time: {res.exec_time_ns} ns")

    offsets_sel = np.stack(
        [res.results[b]["oofs"].reshape(2, H, W).transpose(1, 2, 0) for b in range(B)]
    )
    sizes_sel = np.stack(
        [res.results[b]["osz"].reshape(2, H, W).transpose(1, 2, 0) for b in range(B)]
    )
    weights_sel = np.stack(
        [res.results[b]["owt"].reshape(1, H, W).transpose(1, 2, 0) for b in range(B)]
    )
    pred_cls = np.stack([res.results[b]["opred"][0] for b in range(B)])
    nms_boxes = np.stack([res.results[b]["onmsb"] for b in range(B)])
    nms_scores = np.stack([res.results[b]["onmss"][0] for b in range(B)])
    return offsets_sel, sizes_sel, weights_sel, pred_cls, nms_boxes, nms_scores


# revision 5
# speedup vs baseline: 1.1933x; 1.1061x over previous
"""Trainium2 Bass kernel for nn_DetModel (conv heads + per-sample NMS).

Data-parallel over batch: 8 samples -> 8 NeuronCores. Each core runs
two conv3x3+conv3x3+1x1 branches (f32r matmuls), the small cls branch,
and an exact greedy NMS (rank -> scatter-sort -> chunked IoU ->
fixed-point suppression).
"""
import os
import numpy as np

P = 128
H = W = 256
R = 8
NBLK = H // R
Wp = W + 2
NBOX = 2048
NCH = NBOX // P
NITER = 6
SCORE_THR = 0.05

_BUILT = None


def _install_prof_shim():
    import sys
    import types

    if "antenv.axon_hooks" not in sys.modules:
        from trn_agent_boot.trn_boot import _ntff_profile_via_ctypes

        hook = _ntff_profile_via_ctypes("/opt/axon/libaxon_pjrt.so")
        mod = types.ModuleType("antenv.axon_hooks")
        mod._hook = hook
        mod.get_axon_ntff_profile_hook = lambda: mod._hook
        mod.set_axon_ntff_profile_hook = lambda h: setattr(mod, "_hook", h)
        sys.modules["antenv.axon_hooks"] = mod
    import concourse.bass_utils as bu

    bu.upload_artifacts = lambda tmpdir: f"local:{tmpdir}"


def _build():
    import concourse.bass as bass
    import concourse.bacc as bacc
    import concourse.mybir as mybir
    import concourse.tile as tile

    f32 = mybir.dt.float32
    f32r = mybir.dt.float32r
    bf16 = mybir.dt.bfloat16
    i32 = mybir.dt.int32
    Alu = mybir.AluOpType
    AX = mybir.AxisListType
    Act = mybir.ActivationFunctionType
    M01DT = mybir.dt.float8e4

    nc = bacc.Bacc("TRN2", target_bir_lowering=False, debug=False, num_devices=8)

    featT_d = nc.dram_tensor("featT", [P, H, W], f32r, kind="ExternalInput")
    fclsT_d = nc.dram_tensor("fclsT", [256, 16, 16], f32, kind="ExternalInput")
    wconv_d = nc.dram_tensor("wconv", [4, 9, P, P], f32r, kind="ExternalInput")
    bconv_d = nc.dram_tensor("bconv", [4, P], f32, kind="ExternalInput")
    w1x1_d = nc.dram_tensor("w1x1", [P, 5], f32r, kind="ExternalInput")
    wcls_d = nc.dram_tensor("wcls", [9, 2, P, P], f32, kind="ExternalInput")
    bcls_d = nc.dram_tensor("bcls", [P, 1], f32, kind="ExternalInput")
    fw1_d = nc.dram_tensor("fw1", [P, P], f32, kind="ExternalInput")
    fb1_d = nc.dram_tensor("fb1", [P, 1], f32, kind="ExternalInput")
    fw2_d = nc.dram_tensor("fw2", [P, P], f32, kind="ExternalInput")
    fb2_d = nc.dram_tensor("fb2", [P, 1], f32, kind="ExternalInput")
    ow_d = nc.dram_tensor("ow", [P, 3], f32, kind="ExternalInput")
    ob_d = nc.dram_tensor("ob", [3, 1], f32, kind="ExternalInput")
    i3_d = nc.dram_tensor("i3", [3, 3], f32, kind="ExternalInput")
    boxes_d = nc.dram_tensor("boxes", [NBOX, 4], f32, kind="ExternalInput")
    scores_d = nc.dram_tensor("scores", [1, NBOX], f32, kind="ExternalInput")
    tri_d = nc.dram_tensor("tri", [P, P], f32, kind="ExternalInput")
    trilo_d = nc.dram_tensor("trilo", [P, P], f32, kind="ExternalInput")

    scratch_d = nc.dram_tensor("scratch", [NBOX, 6], f32)

    oofs_d = nc.dram_tensor("oofs", [2, H * W], f32, kind="ExternalOutput")
    osz_d = nc.dram_tensor("osz", [2, H * W], f32, kind="ExternalOutput")
    owt_d = nc.dram_tensor("owt", [1, H * W], f32, kind="ExternalOutput")
    opred_d = nc.dram_tensor("opred", [1, 3], f32, kind="ExternalOutput")
    onmsb_d = nc.dram_tensor("onmsb", [NBOX, 4], f32, kind="ExternalOutput")
    onmss_d = nc.dram_tensor("onmss", [1, NBOX], f32, kind="ExternalOutput")

    NCHK = R * W // 512

    with tile.TileContext(nc) as tc:
        with (
            tc.tile_pool(name="wpool", bufs=1) as wpool,
            tc.tile_pool(name="conv", bufs=1) as conv,
            tc.tile_pool(name="cls", bufs=1) as clsp,
            tc.tile_pool(name="nbig", bufs=1) as nbig,
            tc.tile_pool(name="nsmall", bufs=1) as nsmall,
            tc.tile_pool(name="ntmp", bufs=1) as ntmp,
            tc.tile_pool(name="cps", bufs=3, space="PSUM") as cps,
            tc.tile_pool(name="opsum", bufs=2, space="PSUM") as opsum,
            tc.tile_pool(name="spsum", bufs=2, space="PSUM") as spsum,
        ):
            # ======== consts / conv weights ========
            ZR = wpool.tile([P, 2, Wp], f32)
            nc.gpsimd.memset(ZR[:], 0.0)
            wcr = wpool.tile([P, 4, 9, P], f32r)
            nc.sync.dma_start(wcr[:], wconv_d[:].rearrange("l t ci co -> ci l t co"))
            bc = wpool.tile([P, 4], f32)
            nc.sync.dma_start(bc[:], bconv_d[:].rearrange("l c -> c l"))
            w1r = wpool.tile([P, 5], f32r)
            nc.sync.dma_start(w1r[:], w1x1_d[:])

            FR = [conv.tile([P, R + 4, Wp], f32r, name=f"fr{i}") for i in range(2)]
            X1 = {b: conv.tile([P, R + 2, Wp], f32r, name=f"x1b{b}") for b in (0, 1)}
            X2 = {b: conv.tile([P, R, W], f32r, name=f"x2b{b}") for b in (0, 1)}
            for tl in FR:
                nc.gpsimd.memset(tl[:, :, 0:1].bitcast(f32), 0.0)
                nc.gpsimd.memset(tl[:, :, Wp - 1 : Wp].bitcast(f32), 0.0)
            for b in (0, 1):
                nc.gpsimd.memset(X1[b][:, :, 0:1].bitcast(f32), 0.0)
                nc.gpsimd.memset(X1[b][:, :, Wp - 1 : Wp].bitcast(f32), 0.0)

            # ======== cls branch ========
            fcls = clsp.tile([P, 2, 16, 16], f32)
            nc.sync.dma_start(fcls[:], fclsT_d[:].rearrange("(k ci) a b -> ci k a b", ci=P))
            wcl = clsp.tile([P, 9, 2, P], f32)
            nc.sync.dma_start(wcl[:], wcls_d[:].rearrange("t k ci co -> ci t k co"))
            bcl = clsp.tile([P, 1], f32)
            nc.sync.dma_start(bcl[:], bcls_d[:])
            fw1 = clsp.tile([P, P], f32)
            nc.sync.dma_start(fw1[:], fw1_d[:])
            fb1 = clsp.tile([P, 1], f32)
            nc.sync.dma_start(fb1[:], fb1_d[:])
            fw2 = clsp.tile([P, P], f32)
            nc.sync.dma_start(fw2[:], fw2_d[:])
            fb2 = clsp.tile([P, 1], f32)
            nc.sync.dma_start(fb2[:], fb2_d[:])
            ow = clsp.tile([P, 3], f32)
            nc.sync.dma_start(ow[:], ow_d[:])
            ob = clsp.tile([3, 1], f32)
            nc.sync.dma_start(ob[:], ob_d[:])
            i3 = clsp.tile([3, 3], f32)
            nc.sync.dma_start(i3[:], i3_d[:])

            cls_ps1 = spsum.tile([P, 25], f32, name="sps1", tag="sps")
            for t in range(9):
                dy, dx = t // 3, t % 3
                for k in range(2):
                    nc.tensor.matmul(
                        cls_ps1[:], wcl[:, t, k, :],
                        fcls[:, k, dy : dy + 13 : 3, dx : dx + 13 : 3],
                        start=(t == 0 and k == 0), stop=(t == 8 and k == 1),
                    )
            crelu = clsp.tile([P, 25], f32)
            nc.scalar.activation(crelu[:], cls_ps1[:], Act.Relu, bias=bcl[:])
            gap = clsp.tile([P, 1], f32)
            nc.vector.tensor_reduce(out=gap[:], in_=crelu[:], axis=AX.X, op=Alu.add)
            nc.vector.tensor_scalar(out=gap[:], in0=gap[:], scalar1=1.0 / 25.0, scalar2=None, op0=Alu.mult)
            cls_ps2 = spsum.tile([P, 1], f32, name="sps2", tag="sps")
            nc.tensor.matmul(cls_ps2[:], fw1[:], gap[:], start=True, stop=True)
            h1 = clsp.tile([P, 1], f32)
            nc.scalar.activation(h1[:], cls_ps2[:], Act.Relu, bias=fb1[:])
            cls_ps3 = spsum.tile([P, 1], f32, name="sps3", tag="sps")
            nc.tensor.matmul(cls_ps3[:], fw2[:], h1[:], start=True, stop=True)
            h2 = clsp.tile([P, 1], f32)
            nc.scalar.activation(h2[:], cls_ps3[:], Act.Relu, bias=fb2[:])
            cls_ps4 = spsum.tile([3, 1], f32, name="sps4", tag="sps")
            nc.tensor.matmul(cls_ps4[:], ow[:], h2[:], start=True, stop=True)
            logit = clsp.tile([3, 1], f32)
            nc.vector.tensor_tensor(out=logit[:], in0=cls_ps4[:], in1=ob[:], op=Alu.add)
            cls_ps5 = spsum.tile([1, 3], f32, name="sps5", tag="sps")
            nc.tensor.matmul(cls_ps5[:], logit[:], i3[:], start=True, stop=True)
            lrow = clsp.tile([1, 3], f32)
            nc.vector.tensor_copy(lrow[:], cls_ps5[:])
            lmax = clsp.tile([1, 1], f32)
            nc.vector.tensor_reduce(out=lmax[:], in_=lrow[:], axis=AX.X, op=Alu.max)
            esub = clsp.tile([1, 3], f32)
            nc.vector.tensor_scalar(out=esub[:], in0=lrow[:], scalar1=lmax[:], scalar2=None, op0=Alu.subtract)
            eexp = clsp.tile([1, 3], f32)
            nc.scalar.activation(eexp[:], esub[:], Act.Exp)
            esum = clsp.tile([1, 1], f32)
            nc.vector.tensor_reduce(out=esum[:], in_=eexp[:], axis=AX.X, op=Alu.add)
            einv = clsp.tile([1, 1], f32)
            nc.vector.reciprocal(einv[:], esum[:])
            prob = clsp.tile([1, 3], f32)
            nc.vector.tensor_scalar(out=prob[:], in0=eexp[:], scalar1=einv[:], scalar2=None, op0=Alu.mult)
            nc.scalar.dma_start(opred_d[:], prob[:])

            # ======== NMS part 1: loads, rank, scatter, reload, M01 ========
            s_row = nsmall.tile([1, NBOX], f32)
            nc.sync.dma_start(s_row[:], scores_d[:])
            s_b = nbig.tile([P, NBOX], f32, tag="s_b")
            nc.gpsimd.partition_broadcast(s_b[:], s_row[:])
            s_c = nsmall.tile([P, NCH], f32)
            nc.sync.dma_start(s_c[:], scores_d[0, :].rearrange("(c p) -> p c", p=P))
            tri = nsmall.tile([P, P], f32)
            nc.sync.dma_start(tri[:], tri_d[:])
            tri_m = nsmall.tile([P, P], M01DT)
            nc.vector.tensor_copy(tri_m[:], tri[:])
            trilo = nsmall.tile([P, P], f32)
            nc.sync.dma_start(trilo[:], trilo_d[:])
            trilo_bf = nsmall.tile([P, P], bf16)
            nc.vector.tensor_copy(trilo_bf[:], trilo[:])
            b_c = nsmall.tile([P, NCH, 4], f32)
            nc.sync.dma_start(b_c[:], boxes_d[:].rearrange("(c p) k -> p c k", p=P))

            dy_c = nsmall.tile([P, NCH], f32)
            dx_c = nsmall.tile([P, NCH], f32)
            area_c = nsmall.tile([P, NCH], f32)
            nc.vector.tensor_tensor(out=dy_c[:], in0=b_c[:, :, 2], in1=b_c[:, :, 0], op=Alu.subtract)
            nc.vector.tensor_tensor(out=dx_c[:], in0=b_c[:, :, 3], in1=b_c[:, :, 1], op=Alu.subtract)
            nc.vector.tensor_tensor(out=area_c[:], in0=dy_c[:], in1=dx_c[:], op=Alu.mult)

            gt_c = nsmall.tile([P, NCH], f32)
            tie1_c = nsmall.tile([P, NCH], f32)
            tie2_c = nsmall.tile([P, NCH], f32)
            rank_f = nsmall.tile([P, NCH], f32)
            nc.vector.memset(tie1_c[:], 0.0)
            for c in range(NCH):
                t0 = ntmp.tile([P, NBOX], bf16, name=f"t0_{c}", tag="u0")
                nc.vector.tensor_scalar(
                    out=t0[:], in0=s_b[:], scalar1=s_c[:, c : c + 1], scalar2=None,
                    op0=Alu.is_gt, op1=Alu.add, accum_out=gt_c[:, c : c + 1],
                )
                Wc = (c + 1) * P
                eqt = ntmp.tile([P, NBOX], bf16, name=f"eqt_{c}", tag="u1")
                nc.vector.tensor_scalar(
                    out=eqt[:, :Wc], in0=s_b[:, :Wc], scalar1=s_c[:, c : c + 1],
                    scalar2=None, op0=Alu.is_equal,
                )
                if c > 0:
                    nc.vector.tensor_reduce(
                        out=tie1_c[:, c : c + 1], in_=eqt[:, : c * P], axis=AX.X, op=Alu.add
                    )
                eqw = ntmp.tile([P, P], bf16, name=f"eqw_{c}", tag="eqw")
                nc.vector.tensor_tensor(out=eqw[:], in0=eqt[:, c * P : Wc], in1=trilo_bf[:], op=Alu.mult)
                nc.vector.tensor_reduce(
                    out=tie2_c[:, c : c + 1], in_=eqw[:], axis=AX.X, op=Alu.add
                )
            nc.vector.tensor_tensor(out=rank_f[:], in0=gt_c[:], in1=tie1_c[:], op=Alu.add)
            nc.vector.tensor_tensor(out=rank_f[:], in0=rank_f[:], in1=tie2_c[:], op=Alu.add)
            rank_i = nsmall.tile([P, NCH], i32)
            nc.vector.tensor_copy(rank_i[:], rank_f[:])

            for c in range(NCH):
                st = ntmp.tile([P, 6], f32, name=f"st_{c}", tag="st", bufs=2)
                nc.vector.tensor_copy(st[:, 0:4], b_c[:, c, :])
                nc.vector.tensor_copy(st[:, 4:5], s_c[:, c : c + 1])
                nc.vector.tensor_copy(st[:, 5:6], area_c[:, c : c + 1])
                nc.gpsimd.indirect_dma_start(
                    out=scratch_d[:],
                    out_offset=bass.IndirectOffsetOnAxis(ap=rank_i[:, c : c + 1], axis=0),
                    in_=st[:],
                    in_offset=None,
                )

            y1b = nbig.tile([P, NBOX], f32, tag="y1b")
            x1b = nbig.tile([P, NBOX], f32, tag="x1b")
            y2b = nbig.tile([P, NBOX], f32, tag="y2b")
            x2b = nbig.tile([P, NBOX], f32, tag="x2b")
            arb = nbig.tile([P, NBOX], f32, tag="s_b")  # reuse s_b slot (rank done)
            for t, k in ((y1b, 0), (x1b, 1), (y2b, 2), (x2b, 3), (arb, 5)):
                rr = ntmp.tile([1, NBOX], f32, name=f"rr{k}", tag="rr", bufs=1)
                nc.gpsimd.dma_start(rr[:], scratch_d[:, k].rearrange("(o n) -> o n", o=1))
                nc.gpsimd.partition_broadcast(t[:], rr[:])
            sc_s = nsmall.tile([P, NCH, 6], f32)
            nc.gpsimd.dma_start(sc_s[:], scratch_d[:].rearrange("(c p) k -> p c k", p=P))

            valid_c = nsmall.tile([P, NCH], f32)
            nc.vector.tensor_scalar(
                out=valid_c[:], in0=sc_s[:, :, 4], scalar1=SCORE_THR, scalar2=None, op0=Alu.is_ge
            )

            m01 = []
            for a in range(NCH):
                Wa = NBOX - a * P
                m = nbig.tile([P, Wa], M01DT, name=f"m01_{a}", tag=f"m01_{a}")
                m01.append(m)
                ja = a * P
                u0 = ntmp.tile([P, Wa], f32, name=f"u0_{a}", tag="u0")
                u1 = ntmp.tile([P, Wa], f32, name=f"u1_{a}", tag="u1")
                u2 = ntmp.tile([P, Wa], f32, name=f"u2_{a}", tag="u2")
                nc.vector.tensor_scalar(out=u0[:], in0=y1b[:, ja:], scalar1=sc_s[:, a, 0:1], scalar2=None, op0=Alu.max)
                nc.vector.tensor_scalar(out=u1[:], in0=y2b[:, ja:], scalar1=sc_s[:, a, 2:3], scalar2=None, op0=Alu.min)
                nc.vector.tensor_tensor(out=u0[:], in0=u1[:], in1=u0[:], op=Alu.subtract)
                nc.vector.tensor_scalar(out=u0[:], in0=u0[:], scalar1=0.0, scalar2=None, op0=Alu.max)
                nc.vector.tensor_scalar(out=u1[:], in0=x1b[:, ja:], scalar1=sc_s[:, a, 1:2], scalar2=None, op0=Alu.max)
                nc.vector.tensor_scalar(out=u2[:], in0=x2b[:, ja:], scalar1=sc_s[:, a, 3:4], scalar2=None, op0=Alu.min)
                nc.vector.tensor_tensor(out=u1[:], in0=u2[:], in1=u1[:], op=Alu.subtract)
                nc.vector.tensor_scalar(out=u1[:], in0=u1[:], scalar1=0.0, scalar2=None, op0=Alu.max)
                nc.vector.tensor_tensor(out=u0[:], in0=u0[:], in1=u1[:], op=Alu.mult)
                nc.vector.tensor_scalar(out=u1[:], in0=arb[:, ja:], scalar1=sc_s[:, a, 5:6], scalar2=None, op0=Alu.add)
                nc.vector.tensor_tensor(out=u1[:], in0=u1[:], in1=u0[:], op=Alu.subtract)
                nc.vector.tensor_scalar(out=u1[:], in0=u1[:], scalar1=1e-8, scalar2=0.5, op0=Alu.add, op1=Alu.mult)
                nc.vector.tensor_tensor(out=m[:], in0=u0[:], in1=u1[:], op=Alu.is_gt)
            m01d = []
            for a in range(NCH):
                md = nsmall.tile([P, P], M01DT, name=f"m01d_{a}", tag=f"m01d_{a}")
                m01d.append(md)
                nc.vector.tensor_tensor(out=md[:], in0=m01[a][:, 0:P], in1=tri_m[:], op=Alu.mult)

            # ======== conv branches ========
            from concourse.tile import add_dep_helper
            conv_last_mm = []
            npair1 = (R + 2) // 2
            npair2 = R // 2
            for bi in range(NBLK):
                r0 = bi * R
                g0 = r0 - 2
                Fr = FR[bi % 2]
                lo = max(g0, 0)
                hi = min(g0 + R + 4, H)
                soff = lo - g0
                nc.sync.dma_start(Fr[:, soff : soff + hi - lo, 1 : W + 1], featT_d[:, lo:hi, :])
                if bi == 0:
                    nc.scalar.activation(Fr[:, 0:2, :], ZR[:], Act.Copy)
                if bi == NBLK - 1:
                    nc.scalar.activation(Fr[:, R + 2 : R + 4, :], ZR[:], Act.Copy)

                for br in (0, 1):
                    l1, l2 = (0, 1) if br == 0 else (2, 3)
                    if bi > 0:
                        # x1 rows r0-1, r0 were computed by the previous block
                        # into slots R, R+1 -- slide them down instead of
                        # recomputing (bit-identical values).
                        nc.scalar.activation(
                            X1[br][:, 0:2, :], X1[br][:, R : R + 2, :].bitcast(f32), Act.Copy
                        )
                    for q in range(npair1):
                        if bi > 0 and q == 0:
                            continue
                        pp = cps.tile([P, 2, W], f32, name=f"pp{bi}_{br}_{q}", tag="pp")
                        for t in range(9):
                            dy, dx = t // 3, t % 3
                            nc.tensor.matmul(
                                pp[:], wcr[:, l1, t, :],
                                Fr[:, 2 * q + dy : 2 * q + dy + 2, dx : dx + W],
                                start=(t == 0), stop=(t == 8),
                            )
                        nc.scalar.activation(
                            X1[br][:, 2 * q : 2 * q + 2, 1 : W + 1], pp[:], Act.Relu,
                            bias=bc[:, l1 : l1 + 1],
                        )
                    if bi == 0:
                        nc.scalar.activation(X1[br][:, 0:1, :], ZR[:, 0:1, :], Act.Copy)
                    if bi == NBLK - 1:
                        nc.scalar.activation(X1[br][:, R + 1 : R + 2, :], ZR[:, 0:1, :], Act.Copy)
                    for q in range(npair2):
                        pp = cps.tile([P, 2, W], f32, name=f"pq{bi}_{br}_{q}", tag="pp")
                        for t in range(9):
                            dy, dx = t // 3, t % 3
                            nc.tensor.matmul(
                                pp[:], wcr[:, l2, t, :],
                                X1[br][:, 2 * q + dy : 2 * q + dy + 2, dx : dx + W],
                                start=(t == 0), stop=(t == 8),
                            )
                        nc.scalar.activation(
                            X2[br][:, 2 * q : 2 * q + 2, :], pp[:], Act.Relu,
                            bias=bc[:, l2 : l2 + 1],
                        )
                x2a = X2[0][:].rearrange("p r w -> p (r w)")
                x2bb = X2[1][:].rearrange("p r w -> p (r w)")
                OD = conv.tile([34, R * W], f32, name=f"od{bi}", tag="od", bufs=1)
                for n in range(NCHK):
                    pa = opsum.tile([3, 512], f32, name=f"pa{bi}_{n}", tag="pab")
                    pb = opsum.tile([2, 512], f32, name=f"pb{bi}_{n}", tag="pab")
                    nc.tensor.matmul(pa[:], w1r[:, 0:3], x2a[:, n * 512 : (n + 1) * 512], start=True, stop=True)
                    _mm = nc.tensor.matmul(pb[:], w1r[:, 3:5], x2bb[:, n * 512 : (n + 1) * 512], start=True, stop=True)
                    if n == NCHK - 1:
                        conv_last_mm.append(_mm)
                    nc.scalar.activation(OD[0:3, n * 512 : (n + 1) * 512], pa[:], Act.Copy)
                    nc.scalar.activation(OD[32:34, n * 512 : (n + 1) * 512], pb[:], Act.Copy)
                nc.scalar.dma_start(oofs_d[:, r0 * W : (r0 + R) * W], OD[0:2, :])
                nc.scalar.dma_start(owt_d[:, r0 * W : (r0 + R) * W], OD[2:3, :])
                nc.scalar.dma_start(osz_d[:, r0 * W : (r0 + R) * W], OD[32:34, :])

            # ======== NMS part 2: chunked greedy suppression + outputs ========
            keeps = []
            keep_f = nsmall.tile([P, NCH], f32)
            for b in range(NCH):
                alive = nsmall.tile([P, 1], f32, name=f"alive_{b}", tag=f"alive_{b}")
                if b == 0:
                    nc.vector.tensor_copy(alive[:], valid_c[:, 0:1])
                else:
                    sup_ps = spsum.tile([P, 1], f32, name=f"cps_{b}", tag="sps")
                    for a in range(b):
                        _smm = nc.tensor.matmul(
                            sup_ps[:], m01[a][:, (b - a) * P : (b - a + 1) * P], keeps[a][:],
                            start=(a == 0), stop=(a == b - 1),
                        )
                        if a == 0:
                            add_dep_helper(
                                _smm.ins, conv_last_mm[min(14 + b, NBLK - 2)].ins,
                                sync=False, reason="interleave nms after conv",
                            )
                    nc.vector.tensor_scalar(
                        out=alive[:], in0=sup_ps[:], scalar1=0.0, scalar2=valid_c[:, b : b + 1],
                        op0=Alu.is_equal, op1=Alu.mult,
                    )
                kb = nsmall.tile([P, 1], M01DT, name=f"keep_{b}", tag=f"keep_{b}")
                keeps.append(kb)
                nc.vector.tensor_copy(kb[:], alive[:])
                for t in range(NITER):
                    ips = spsum.tile([P, 1], f32, name=f"ips_{b}_{t}", tag="sps")
                    _imm = nc.tensor.matmul(ips[:], m01d[b][:], kb[:], start=True, stop=True)
                    if b == 0 and t == 0:
                        add_dep_helper(
                            _imm.ins, conv_last_mm[14].ins,
                            sync=False, reason="interleave nms after conv",
                        )
                    nc.vector.tensor_scalar(
                        out=kb[:], in0=ips[:], scalar1=0.0, scalar2=alive[:],
                        op0=Alu.is_equal, op1=Alu.mult,
                    )
                nc.vector.tensor_copy(keep_f[:, b : b + 1], kb[:])

            for c in range(NCH):
                obx = ntmp.tile([P, 4], f32, name=f"obx_{c}", tag="obx", bufs=2)
                osc = ntmp.tile([P, 1], f32, name=f"osc_{c}", tag="osc", bufs=2)
                nc.vector.tensor_scalar(
                    out=obx[:], in0=sc_s[:, c, 0:4], scalar1=keep_f[:, c : c + 1],
                    scalar2=None, op0=Alu.mult,
                )
                nc.vector.tensor_scalar(
                    out=osc[:], in0=sc_s[:, c, 4:5], scalar1=keep_f[:, c : c + 1],
                    scalar2=None, op0=Alu.mult,
                )
                nc.gpsimd.dma_start(onmsb_d[:].rearrange("(c p) k -> p k c", p=P)[:, :, c], obx[:])
                nc.gpsimd.dma_start(onmss_d[0, :].rearrange("(c p) -> p c", p=P)[:, c : c + 1], osc[:])

    nc.compile()
    return nc


def kernel(feat_reg, feat_cls, cls, boxes, scores, params):
    global _BUILT
    trace = os.environ.get("KERNEL_TRACE", "0") == "1"
    if trace:
        _install_prof_shim()
    from concourse.bass_utils import run_bass_kernel_spmd

    feat_reg = np.asarray(feat_reg, dtype=np.float32)
    feat_cls = np.asarray(feat_cls, dtype=np.float32)
    cls = np.asarray(cls, dtype=np.int32)
    boxes = np.asarray(boxes, dtype=np.float32)
    scores = np.asarray(scores, dtype=np.float32)
    pr = {k: np.asarray(v, dtype=np.float32) for k, v in params.items()}

    if _BUILT is None:
        _BUILT = _build()
    nc = _BUILT

    B = feat_reg.shape[0]
    wconv = np.stack(
        [pr["reg_w1"], pr["reg_w2"], pr["sz_w1"], pr["sz_w2"]]
    ).reshape(4, 9, P, P).astype(np.float32)
    bconv = np.stack(
        [pr["reg_b1"], pr["reg_b2"], pr["sz_b1"], pr["sz_b2"]]
    ).astype(np.float32)
    wcls = pr["cls_cw"].reshape(9, 256, P).reshape(9, 2, P, P).astype(np.float32)
    tri = (np.arange(P)[:, None] < np.arange(P)[None, :]).astype(np.float32)

    in_maps = []
    for b in range(B):
        c = int(cls[b])
        w1x1 = np.concatenate(
            [pr["ofs_w"][:, 2 * c : 2 * c + 2], pr["wt_w"][:, c : c + 1],
             pr["size_w"][:, 2 * c : 2 * c + 2]], axis=1
        ).astype(np.float32)
        in_maps.append({
            "featT": np.ascontiguousarray(feat_reg[b].transpose(2, 0, 1)),
            "fclsT": np.ascontiguousarray(feat_cls[b].transpose(2, 0, 1)),
            "wconv": wconv, "bconv": bconv, "w1x1": w1x1,
            "wcls": wcls, "bcls": pr["cls_cb"].reshape(P, 1),
            "fw1": pr["cls_fw1"], "fb1": pr["cls_fb1"].reshape(P, 1),
            "fw2": pr["cls_fw2"], "fb2": pr["cls_fb2"].reshape(P, 1),
            "ow": pr["cls_ow"], "ob": pr["cls_ob"].reshape(3, 1),
            "i3": np.eye(3, dtype=np.float32),
            "boxes": boxes[b], "scores": scores[b].reshape(1, NBOX),
            "tri": tri, "trilo": np.ascontiguousarray(tri.T),
        })

    res = run_bass_kernel_spmd(nc, in_maps, list(range(8)), trace=trace)
    if trace and res.exec_time_ns is not None:
        print(f"HW exec time: {res.exec_time_ns} ns")

    offsets_sel = np.stack(
        [res.results[b]["oofs"].reshape(2, H, W).transpose(1, 2, 0) for b in range(B)]
    )
    sizes_sel = np.stack(
        [res.results[b]["osz"].reshape(2, H, W).transpose(1, 2, 0) for b in range(B)]
    )
    weights_sel = np.stack(
        [res.results[b]["owt"].reshape(1, H, W).transpose(1, 2, 0) for b in range(B)]
    )
    pred_cls = np.stack([res.results[b]["opred"][0] for b in range(B)])
    nms_boxes = np.stack([res.results[b]["onmsb"] for b in range(B)])
    nms_scores = np.stack([res.results[b]["onmss"][0] for b in range(B)])
    return offsets_sel, sizes_sel, weights_sel, pred_cls, nms_boxes, nms_scores
